# revision 11
# baseline (speedup 1.0000x reference)
"""MinGRU cell on 8 Trainium2 NeuronCores.

Math: the reference computes, per (batch b, hidden channel j), the linear
recurrence (written there in log-space for stability)

    h_t = c_t * h_{t-1} + v_t,      h_0 = g(h0)
    c_t = 1 - sigmoid(kz_t) = sigmoid(-kz_t)
    v_t = sigmoid(kz_t) * g(kh_t)
    kz = x @ Wz^T + bz,  kh = x @ Wh^T + bh
    g(u) = u + 0.5 if u >= 0 else sigmoid(u) = max(sigmoid(u), u + 0.5)
    (the max identity is exact: u + 0.5 - sigmoid(u) is 0 at u=0 and increasing)

All quantities are positive and O(1) (h_t is a convex combination), so the
linear-space recurrence is numerically fine in fp32 (~7e-4 max rel err vs
the log-space reference).

Sharding: data-parallel over batch, one batch row per core (B == 8 cores).
Weights replicated. Each core computes kz/kh with the tensor engine in a
[h-partition, s-free] layout so the recurrence runs as a single
tensor_tensor_scan per (h-tile, s-block) along the free axis, chained via
the previous block's last column.

Host-side layout only (no math): x is fed pre-transposed (D, S) per batch so
the contraction dim lands on partitions; output comes back (H, S) and is
transposed on the host.
"""

import numpy as np

import concourse.bass as bass
import concourse.mybir as mybir
import concourse.tile as tile
from concourse import bacc
from concourse.bass_utils import run_bass_kernel_spmd

B, S, D, H = 8, 4096, 1024, 1024
N_CORES = 8
P = 128              # partitions
SB = 512             # s-block (columns per matmul / PSUM bank)
NSB = S // SB        # 8
DT = D // P          # 8 contraction tiles
HT = H // P          # 8 hidden tiles

F32 = mybir.dt.float32
# Matmul operand dtype: float32 (exact, 4 cyc/row) or float32r (1 cyc/row).
MM_DT = mybir.dt.float32

_CACHE = {}


def _build_program():
    nc = bacc.Bacc(trn_type="TRN2")

    xT = nc.dram_tensor("xt", [D, S], MM_DT, kind="ExternalInput")
    wzT = nc.dram_tensor("wzt", [D, H], MM_DT, kind="ExternalInput")
    whT = nc.dram_tensor("wht", [D, H], MM_DT, kind="ExternalInput")
    bzg = nc.dram_tensor("bzg", [P, HT], F32, kind="ExternalInput")
    bhg = nc.dram_tensor("bhg", [P, HT], F32, kind="ExternalInput")
    h0g = nc.dram_tensor("h0g", [P, HT], F32, kind="ExternalInput")
    hT = nc.dram_tensor("ht", [H, S], F32, kind="ExternalOutput")

    AF = mybir.ActivationFunctionType
    OP = mybir.AluOpType

    with tile.TileContext(nc) as tc:
        with (
            tc.tile_pool(name="wpool", bufs=1) as wpool,
            tc.tile_pool(name="bias", bufs=1) as bias,
            tc.tile_pool(name="xin", bufs=3) as xin,
            tc.tile_pool(name="psz", bufs=2, space="PSUM") as psz,
            tc.tile_pool(name="psh", bufs=2, space="PSUM") as psh,
            tc.tile_pool(name="inter", bufs=3) as inter,
            tc.tile_pool(name="outp", bufs=2) as outp,
        ):
            # Weights, laid out [p(d-in-tile), d-tile, h]
            wz_sb = wpool.tile([P, DT, H], MM_DT, tag="wz")
            nc.sync.dma_start(
                out=wz_sb, in_=wzT.ap().rearrange("(dt p) h -> p dt h", p=P)
            )
            wh_sb = wpool.tile([P, DT, H], MM_DT, tag="wh")
            nc.sync.dma_start(
                out=wh_sb, in_=whT.ap().rearrange("(dt p) h -> p dt h", p=P)
            )

            # Bias / initial-state columns, [p(h-in-tile), h-tile]
            bz_sb = bias.tile([P, HT], F32, tag="bz")
            nc.sync.dma_start(out=bz_sb, in_=bzg.ap())
            bh_sb = bias.tile([P, HT], F32, tag="bh")
            nc.sync.dma_start(out=bh_sb, in_=bhg.ap())
            h0_sb = bias.tile([P, HT], F32, tag="h0")
            nc.sync.dma_start(out=h0_sb, in_=h0g.ap())

            nbz_sb = bias.tile([P, HT], F32, tag="nbz")
            nc.vector.tensor_scalar_mul(nbz_sb[:], bz_sb[:], -1.0)
            bhh_sb = bias.tile([P, HT], F32, tag="bhh")  # bh + 0.5
            nc.vector.tensor_scalar_add(bhh_sb[:], bh_sb[:], 0.5)
            neg_half = bias.tile([P, 1], F32, tag="nhalf")
            nc.vector.memset(neg_half[:], -0.5)

            # g0 = max(sigmoid(h0), h0 + 0.5)
            g0_s = bias.tile([P, HT], F32, tag="g0s")
            nc.scalar.activation(g0_s[:], h0_sb[:], AF.Sigmoid)
            g0_t = bias.tile([P, HT], F32, tag="g0t")
            nc.vector.tensor_scalar_add(g0_t[:], h0_sb[:], 0.5)
            g0 = bias.tile([P, HT], F32, tag="g0")
            nc.vector.tensor_max(g0[:], g0_s[:], g0_t[:])

            xT_v = xT.ap().rearrange("(dt p) s -> p dt s", p=P)
            hT_v = hT.ap().rearrange("(ht p) s -> p ht s", p=P)

            prev_out = [None] * HT
            for sb in range(NSB):
                x_t = xin.tile([P, DT, SB], MM_DT, tag="x")
                nc.sync.dma_start(out=x_t, in_=xT_v[:, :, sb * SB:(sb + 1) * SB])

                for hi in range(HT):
                    kz = psz.tile([P, SB], F32)
                    kh = psh.tile([P, SB], F32)
                    for di in range(DT):
                        nc.tensor.matmul(
                            kz[:],
                            wz_sb[:, di, hi * P:(hi + 1) * P],
                            x_t[:, di, :],
                            start=(di == 0),
                            stop=(di == DT - 1),
                        )
                    for di in range(DT):
                        nc.tensor.matmul(
                            kh[:],
                            wh_sb[:, di, hi * P:(hi + 1) * P],
                            x_t[:, di, :],
                            start=(di == 0),
                            stop=(di == DT - 1),
                        )

                    bcol = (hi, hi + 1)
                    zt = inter.tile([P, SB], F32, tag="z")
                    nc.scalar.activation(
                        zt[:], kz[:], AF.Sigmoid, bias=bz_sb[:, bcol[0]:bcol[1]]
                    )
                    ct = inter.tile([P, SB], F32, tag="c")
                    nc.scalar.activation(
                        ct[:], kz[:], AF.Sigmoid,
                        bias=nbz_sb[:, bcol[0]:bcol[1]], scale=-1.0,
                    )
                    # tilde = g(kh + bh) = max(sigmoid(kh + bh), kh + bh + 0.5)
                    # m = kh + bh + 0.5 ; a = sigmoid(m - 0.5) ; tilde = max(a, m)
                    mt = inter.tile([P, SB], F32, tag="m")
                    nc.scalar.activation(
                        mt[:], kh[:], AF.Identity, bias=bhh_sb[:, bcol[0]:bcol[1]]
                    )
                    at = inter.tile([P, SB], F32, tag="a")
                    nc.scalar.activation(at[:], mt[:], AF.Sigmoid, bias=neg_half[:])
                    tl = inter.tile([P, SB], F32, tag="tl")
                    nc.vector.tensor_max(tl[:], at[:], mt[:])
                    vt = inter.tile([P, SB], F32, tag="v")
                    nc.vector.tensor_mul(vt[:], zt[:], tl[:])

                    ot = outp.tile([P, SB], F32, tag=f"o{hi}")
                    init = (
                        g0[:, hi:hi + 1] if sb == 0
                        else prev_out[hi][:, SB - 1:SB]
                    )
                    nc.vector.tensor_tensor_scan(
                        ot[:], ct[:], vt[:], init, op0=OP.mult, op1=OP.add
                    )
                    prev_out[hi] = ot
                    nc.sync.dma_start(
                        out=hT_v[:, hi, sb * SB:(sb + 1) * SB], in_=ot[:]
                    )
    nc.finalize()
    return nc


def _get_program():
    if "nc" not in _CACHE:
        _CACHE["nc"] = _build_program()
    return _CACHE["nc"]


def run(inputs, **kw):
    """Run on hardware; returns (output (B,S,H) fp32, BassKernelResults)."""
    x = np.asarray(inputs["x"], dtype=np.float32)
    h0 = np.asarray(inputs["h0"], dtype=np.float32)
    Wz = np.asarray(inputs["Wz"], dtype=np.float32)
    bz = np.asarray(inputs["bz"], dtype=np.float32)
    Wh = np.asarray(inputs["Wh"], dtype=np.float32)
    bh = np.asarray(inputs["bh"], dtype=np.float32)

    wzT = np.ascontiguousarray(Wz.T)
    whT = np.ascontiguousarray(Wh.T)
    bzg = np.ascontiguousarray(bz.reshape(HT, P).T)
    bhg = np.ascontiguousarray(bh.reshape(HT, P).T)

    in_maps = []
    for b in range(N_CORES):
        in_maps.append({
            "xt": np.ascontiguousarray(x[b].T),
            "wzt": wzT,
            "wht": whT,
            "bzg": bzg,
            "bhg": bhg,
            "h0g": np.ascontiguousarray(h0[b, 0].reshape(HT, P).T),
        })

    nc = _get_program()
    res = run_bass_kernel_spmd(nc, in_maps, core_ids=list(range(N_CORES)), **kw)
    out = np.stack([res.results[b]["ht"].T for b in range(N_CORES)], axis=0)
    return np.ascontiguousarray(out), res


def kernel(**inputs):
    out, _ = run(inputs)
    return out


# revision 23
# speedup vs baseline: 113.2790x; 113.2790x over previous
"""MinGRU cell on 8 Trainium2 NeuronCores.

Math: the reference computes, per (batch b, hidden channel j), the linear
recurrence (written there in log-space for stability)

    h_t = c_t * h_{t-1} + v_t,      h_0 = g(h0)
    c_t = 1 - sigmoid(kz_t) = sigmoid(-kz_t)
    v_t = sigmoid(kz_t) * g(kh_t)
    kz = x @ Wz^T + bz,  kh = x @ Wh^T + bh
    g(u) = u + 0.5 if u >= 0 else sigmoid(u) = max(sigmoid(u), u + 0.5)
    (the max identity is exact: u + 0.5 - sigmoid(u) is 0 at u=0 and increasing)

All quantities are positive and O(1) (h_t is a convex combination), so the
linear-space recurrence is numerically fine in fp32 (~7e-4 max rel err vs
the log-space reference).

Sharding: data-parallel over batch, one batch row per core (B == 8 cores).
Weights replicated. Each core computes kz/kh with the tensor engine in a
[h-partition, s-free] layout so the recurrence runs as a single
tensor_tensor_scan per (h-tile, s-block) along the free axis, chained via
the previous block's last column.

Host-side layout only (no math): x is fed pre-transposed (D, S) per batch so
the contraction dim lands on partitions; output comes back (H, S) and is
transposed on the host.
"""

import numpy as np

import concourse.bass as bass
import concourse.mybir as mybir
import concourse.tile as tile
from concourse import bacc
from concourse.bass_utils import run_bass_kernel_spmd

B, S, D, H = 8, 4096, 1024, 1024
N_CORES = 8
P = 128              # partitions
SB = 512             # s-block (columns per matmul / PSUM bank)
NSB = S // SB        # 8
DT = D // P          # 8 contraction tiles
HT = H // P          # 8 hidden tiles

F32 = mybir.dt.float32
# Matmul operand dtype: float32 (exact, 4 cyc/row) or float32r (1 cyc/row).
MM_DT = mybir.dt.float32r

_CACHE = {}


def _build_program(ablate=(), repeat=1, mm_dt=None, bufs=None):
    """ablate: subset of {'mm','act','dve','scan','outdma','xdma'} to stub out.
    repeat: unroll the whole body N times (timing only; results identical)."""
    if mm_dt is None:
        mm_dt = MM_DT
    bufs = {**{"xin": 3, "psz": 2, "psh": 2, "inter": 3, "outp": 2}, **(bufs or {})}
    nc = bacc.Bacc(trn_type="TRN2")

    xT = nc.dram_tensor("xt", [D, S], mm_dt, kind="ExternalInput")
    wzT = nc.dram_tensor("wzt", [D, H], mm_dt, kind="ExternalInput")
    whT = nc.dram_tensor("wht", [D, H], mm_dt, kind="ExternalInput")
    bzg = nc.dram_tensor("bzg", [P, HT], F32, kind="ExternalInput")
    bhg = nc.dram_tensor("bhg", [P, HT], F32, kind="ExternalInput")
    h0g = nc.dram_tensor("h0g", [P, HT], F32, kind="ExternalInput")
    hT = nc.dram_tensor("ht", [H, S], F32, kind="ExternalOutput")

    AF = mybir.ActivationFunctionType
    OP = mybir.AluOpType

    with tile.TileContext(nc) as tc:
        with (
            tc.tile_pool(name="wpool", bufs=1) as wpool,
            tc.tile_pool(name="bias", bufs=1) as bias,
            tc.tile_pool(name="xin", bufs=bufs["xin"]) as xin,
            tc.tile_pool(name="psz", bufs=bufs["psz"], space="PSUM") as psz,
            tc.tile_pool(name="psh", bufs=bufs["psh"], space="PSUM") as psh,
            tc.tile_pool(name="inter", bufs=bufs["inter"]) as inter,
            tc.tile_pool(name="outp", bufs=bufs["outp"]) as outp,
        ):
            # Weights, laid out [p(d-in-tile), d-tile, h]; chunked per d-tile
            # so the first matmuls can start before all weights land.
            wz_sb = wpool.tile([P, DT, H], mm_dt, tag="wz")
            wh_sb = wpool.tile([P, DT, H], mm_dt, tag="wh")
            wzT_v = wzT.ap().rearrange("(dt p) h -> p dt h", p=P)
            whT_v = whT.ap().rearrange("(dt p) h -> p dt h", p=P)
            for di in range(DT):
                nc.sync.dma_start(
                    out=wz_sb[:, di:di + 1, :], in_=wzT_v[:, di:di + 1, :]
                )
                nc.sync.dma_start(
                    out=wh_sb[:, di:di + 1, :], in_=whT_v[:, di:di + 1, :]
                )

            # Bias / initial-state columns, [p(h-in-tile), h-tile]
            bz_sb = bias.tile([P, HT], F32, tag="bz")
            nc.sync.dma_start(out=bz_sb, in_=bzg.ap())
            bh_sb = bias.tile([P, HT], F32, tag="bh")
            nc.sync.dma_start(out=bh_sb, in_=bhg.ap())
            h0_sb = bias.tile([P, HT], F32, tag="h0")
            nc.sync.dma_start(out=h0_sb, in_=h0g.ap())

            nbz_sb = bias.tile([P, HT], F32, tag="nbz")
            nc.vector.tensor_scalar_mul(nbz_sb[:], bz_sb[:], -1.0)
            bhh_sb = bias.tile([P, HT], F32, tag="bhh")  # bh + 0.5
            nc.vector.tensor_scalar_add(bhh_sb[:], bh_sb[:], 0.5)
            neg_half = bias.tile([P, 1], F32, tag="nhalf")
            nc.vector.memset(neg_half[:], -0.5)

            # g0 = max(sigmoid(h0), h0 + 0.5)
            g0_s = bias.tile([P, HT], F32, tag="g0s")
            nc.scalar.activation(g0_s[:], h0_sb[:], AF.Sigmoid)
            g0_t = bias.tile([P, HT], F32, tag="g0t")
            nc.vector.tensor_scalar_add(g0_t[:], h0_sb[:], 0.5)
            g0 = bias.tile([P, HT], F32, tag="g0")
            nc.vector.tensor_max(g0[:], g0_s[:], g0_t[:])

            xT_v = xT.ap().rearrange("(dt p) s -> p dt s", p=P)
            hT_v = hT.ap().rearrange("(ht p) s -> p ht s", p=P)

            for _rep in range(repeat):
              prev_out = [None] * HT
              for sb in range(NSB):
                x_t = xin.tile([P, DT, SB], mm_dt, tag="x")
                if "xdma" not in ablate:
                    nc.sync.dma_start(
                        out=x_t, in_=xT_v[:, :, sb * SB:(sb + 1) * SB]
                    )

                for hi in range(HT):
                    kz = psz.tile([P, SB], F32)
                    kh = psh.tile([P, SB], F32)
                    if "mm" not in ablate:
                        for di in range(DT):
                            nc.tensor.matmul(
                                kz[:],
                                wz_sb[:, di, hi * P:(hi + 1) * P],
                                x_t[:, di, :],
                                start=(di == 0),
                                stop=(di == DT - 1),
                            )
                        for di in range(DT):
                            nc.tensor.matmul(
                                kh[:],
                                wh_sb[:, di, hi * P:(hi + 1) * P],
                                x_t[:, di, :],
                                start=(di == 0),
                                stop=(di == DT - 1),
                            )

                    bcol = (hi, hi + 1)
                    zt = inter.tile([P, SB], F32, tag="z")
                    ct = inter.tile([P, SB], F32, tag="c")
                    mt = inter.tile([P, SB], F32, tag="m")
                    at = inter.tile([P, SB], F32, tag="a")
                    if "act" not in ablate:
                        nc.scalar.activation(
                            zt[:], kz[:], AF.Sigmoid, bias=bz_sb[:, bcol[0]:bcol[1]]
                        )
                        nc.scalar.activation(
                            ct[:], kz[:], AF.Sigmoid,
                            bias=nbz_sb[:, bcol[0]:bcol[1]], scale=-1.0,
                        )
                        # tilde = g(kh+bh) = max(sigmoid(kh+bh), kh+bh+0.5)
                        # m = kh+bh+0.5 ; a = sigmoid(m-0.5) ; tilde = max(a, m)
                        nc.scalar.activation(
                            mt[:], kh[:], AF.Identity,
                            bias=bhh_sb[:, bcol[0]:bcol[1]],
                        )
                        nc.scalar.activation(
                            at[:], mt[:], AF.Sigmoid, bias=neg_half[:]
                        )
                    tl = inter.tile([P, SB], F32, tag="tl")
                    vt = inter.tile([P, SB], F32, tag="v")
                    if "dve" not in ablate:
                        nc.vector.tensor_max(tl[:], at[:], mt[:])
                        nc.vector.tensor_mul(vt[:], zt[:], tl[:])

                    ot = outp.tile([P, SB], F32, tag=f"o{hi}")
                    if "scan" not in ablate:
                        init = (
                            g0[:, hi:hi + 1] if sb == 0
                            else prev_out[hi][:, SB - 1:SB]
                        )
                        nc.vector.tensor_tensor_scan(
                            ot[:], ct[:], vt[:], init, op0=OP.mult, op1=OP.add
                        )
                        prev_out[hi] = ot
                    if "outdma" not in ablate:
                        nc.sync.dma_start(
                            out=hT_v[:, hi, sb * SB:(sb + 1) * SB], in_=ot[:]
                        )
    nc.finalize()
    return nc


def _get_program():
    if "nc" not in _CACHE:
        _CACHE["nc"] = _build_program()
    return _CACHE["nc"]


def run(inputs, **kw):
    """Run on hardware; returns (output (B,S,H) fp32, BassKernelResults)."""
    x = np.asarray(inputs["x"], dtype=np.float32)
    h0 = np.asarray(inputs["h0"], dtype=np.float32)
    Wz = np.asarray(inputs["Wz"], dtype=np.float32)
    bz = np.asarray(inputs["bz"], dtype=np.float32)
    Wh = np.asarray(inputs["Wh"], dtype=np.float32)
    bh = np.asarray(inputs["bh"], dtype=np.float32)

    mm_np = mybir.dt.np(MM_DT)
    wzT = np.ascontiguousarray(Wz.T).astype(mm_np)
    whT = np.ascontiguousarray(Wh.T).astype(mm_np)
    bzg = np.ascontiguousarray(bz.reshape(HT, P).T)
    bhg = np.ascontiguousarray(bh.reshape(HT, P).T)

    in_maps = []
    for b in range(N_CORES):
        in_maps.append({
            "xt": np.ascontiguousarray(x[b].T).astype(mm_np),
            "wzt": wzT,
            "wht": whT,
            "bzg": bzg,
            "bhg": bhg,
            "h0g": np.ascontiguousarray(h0[b, 0].reshape(HT, P).T),
        })

    nc = _get_program()
    res = run_bass_kernel_spmd(nc, in_maps, core_ids=list(range(N_CORES)), **kw)
    out = np.stack([res.results[b]["ht"].T for b in range(N_CORES)], axis=0)
    return np.ascontiguousarray(out), res


def kernel(**inputs):
    out, _ = run(inputs)
    return out
